# revision 5
# baseline (speedup 1.0000x reference)
"""Trainium2 Bass kernel for the blocked-DCT corner-mask layer.

Math: for each 8x8 block B of the image, the reference computes
    coeffs = D^T B D        (2D DCT-II)
    out_c  = D (coeffs * mask_c) D^T   for 4 corner masks c
Each mask is an outer product of half-indicators, so with
    L = D[:, :4] @ D[:, :4].T   (symmetric projection),  H = I - L
the whole pipeline collapses to
    out_0 = L B L,  out_1 = L B H,  out_2 = H B L,  out_3 = H B H.

Per-8-row/8-col application over a full 512x512 image is multiplication by
the 128x128 block-diagonal BDL = blockdiag(L x 16) (symmetric) on either
side.  On-chip per [128, 512] tile X:
    A-mm  c: lhsT = X[:, 128c:128c+128]  -> A(c) = [R^T(c) | RH^T(c)],
             R = BDL @ X, RH = BDH @ X            (PE, rhs=[BDL|BDH], N=256)
    out-mm c: lhsT = R^T(c)  -> [O0(c) | O1(c)];
              lhsT = RH^T(c) -> [O2(c) | O3(c)]   (PE, N=256)

All HBM I/O and matmul operands are bf16 (tolerance is 2e-2; bf16 costs
~3e-3): halves both the DMA bytes (the f32 bottleneck) and PE time.
The f32->bf16 cast rides the PSUM->SBUF copies that are needed anyway.

Pipelining: PSUM is two rings of [128,1024] f32 (2 banks) x 2 bufs --
front ring (A) and back ring (p01/p23 alternating), so tile i+1's
matmuls overlap tile i's drain copies.  Each PSUM drain is a single
wide copy op (DVE: A-head + [O0|O1]; ACT: A-tail + [O2|O3]), with the
de-interleave folded into the copy access pattern.  Outputs stage into
per-corner [128, 512*4] buffers covering a whole image plane, so each
in/out DMA moves 512 KB (4 tiles) in one descriptor batch.

Sharding: data-parallel over batch, 4 batches (12 images) per core.
"""

import numpy as np

FULL_B, DCH, H, W = 32, 3, 512, 512
N_CORES = 8
B_PER_CORE = FULL_B // N_CORES       # 4
IMGS = B_PER_CORE * DCH              # 12 images per core
P = 128
SPLIT = 416  # DVE/ACT balance point for the A-copy (DVE 1.042, ACT 0.833 ns/col)

_BUILT = {}


def _consts() -> np.ndarray:
    """[128, 256] = [BDL | BDH] constants, computed in float64 -> bf16."""
    import ml_dtypes

    N = 8
    x = np.arange(N, dtype=np.float64)[:, None]
    u = np.arange(N, dtype=np.float64)[None, :]
    alpha = np.full(N, np.sqrt(2.0 / N))
    alpha[0] = np.sqrt(1.0 / N)
    D = alpha[None, :] * np.cos(np.pi * u * (2.0 * x + 1.0) / (2.0 * N))
    L = D[:, :4] @ D[:, :4].T
    Hm = np.eye(N) - L
    BDL = np.kron(np.eye(16), L)
    BDH = np.kron(np.eye(16), Hm)
    cst = np.concatenate([BDL, BDH], axis=1)
    return np.ascontiguousarray(cst.astype(ml_dtypes.bfloat16))


def _img_ap(dram_ap, row0):
    """[128, 4, 512] view of one 512-row image plane: (t*128+p, w) -> (p, t, w)."""
    return dram_ap[row0 : row0 + 512, :].rearrange("(t p) w -> p t w", t=4)


def _body(ctx, tc, o_ap, x_ap, c_ap, n_imgs):
    import concourse.mybir as mybir

    nc = tc.nc
    f32 = mybir.dt.float32
    bf16 = mybir.dt.bfloat16

    cpool = ctx.enter_context(tc.tile_pool(name="const", bufs=1))
    cst = cpool.tile([P, 256], bf16)
    nc.sync.dma_start(cst[:], c_ap[:, :])
    BDLH = cst[:, 0:256]  # packed [BDL | BDH] rhs, N=256

    sb = ctx.enter_context(tc.tile_pool(name="sb", bufs=1))
    ps = ctx.enter_context(tc.tile_pool(name="ps", bufs=1, space="PSUM"))

    xbigs = {}
    obigs = {}

    def front(i):
        """per-image input DMA + row-transform matmuls A = x^T @ [BDL|BDH] + copy.

        A-mm for chunk c: lhsT = X[:, 128c:128c+128] (contraction over
        image rows) -> out [128 = col-in-chunk, 256] = [R^T(c) | RH^T(c)]
        where R = BDL @ X, RH = BDH @ X.  No identity transposes needed.
        """
        img, t = divmod(i, 4)
        if t == 0:
            xbig = sb.tile([P, 2048], bf16, tag="x", bufs=3, name=f"x_{img}")
            xv = xbig[:].rearrange("p (t w) -> p t w", t=4)
            nc.gpsimd.dma_start(xv, _img_ap(x_ap, img * 512))  # SWDGE ring
            xbigs[img] = xbig
        xbig = xbigs[img]

        a_ps = ps.tile([P, 1024], f32, tag="aps", bufs=2, name=f"aps_{i}")
        for c in range(4):
            nc.tensor.matmul(
                a_ps[:, 256 * c : 256 * (c + 1)],
                lhsT=xbig[:, 512 * t + 128 * c : 512 * t + 128 * (c + 1)],
                rhs=BDLH,
                start=True,
                stop=True,
            )
        # split the A drain across both engines (balanced by engine rate)
        a_sb = sb.tile([P, 1024], bf16, tag="as", bufs=4, name=f"a_{i}")
        nc.vector.tensor_copy(a_sb[:, 0:SPLIT], a_ps[:, 0:SPLIT])  # DVE
        nc.scalar.copy(a_sb[:, SPLIT:1024], a_ps[:, SPLIT:1024])  # ACT
        return a_sb

    def back_half(i, a_sb, half):
        """One back half-phase: 4 matmuls -> [128,1024] PSUM -> one wide
        de-interleaving copy into the per-image staging tiles -> (at t=3)
        two 512 KB output DMAs."""
        img, t = divmod(i, 4)
        a_v = a_sb[:].rearrange("p (c s l) -> p c s l", c=4, s=2, l=128)

        pb = ps.tile([P, 1024], f32, tag="pb", bufs=2, name=f"p{half}_{i}")
        for c in range(4):
            nc.tensor.matmul(
                pb[:, 256 * c : 256 * (c + 1)],
                lhsT=a_v[:, c, half, :],
                rhs=BDLH,
                start=True,
                stop=True,
            )  # [O_{2h}(c) | O_{2h+1}(c)]

        osb = sb.tile([P, 1024], bf16, tag=f"o{half}", bufs=4, name=f"o{half}_{i}")
        # de-interleave (c s l) -> (s c l) while draining PSUM, one wide op
        src = pb[:].rearrange("p (c s l) -> p s c l", c=4, s=2, l=128)
        dst = osb[:].rearrange("p (s c l) -> p s c l", s=2, c=4, l=128)
        if half == 0:
            nc.vector.tensor_copy(dst, src)  # DVE
        else:
            nc.scalar.copy(dst, src)  # ACT

        # per-tile output DMAs: fire as soon as the drain lands (keeps the
        # DMA rings continuously fed instead of bursting per image).  All
        # triggers go through the otherwise-idle Sync sequencer so the
        # copy engines never stall behind a DMA-trigger sem wait.
        eng = nc.sync
        for s in range(2):
            ci = 2 * half + s
            orow = (ci * n_imgs + img) * 512 + t * 128
            eng.dma_start(o_ap[orow : orow + 128, :], osb[:, 512 * s : 512 * (s + 1)])

    # one-stage software skew: tile i's output stages are emitted after
    # tile i+1's front stage, keeping PE fed while PSUM banks drain
    ntiles = n_imgs * 4
    pending = None
    for i in range(ntiles):
        cch = front(i)
        if pending is not None:
            back_half(i - 1, pending, 0)
            back_half(i - 1, pending, 1)
        pending = cch
    back_half(ntiles - 1, pending, 0)
    back_half(ntiles - 1, pending, 1)


def _build(n_imgs=IMGS):
    key = n_imgs
    if key in _BUILT:
        return _BUILT[key]
    from contextlib import ExitStack

    import concourse.bacc as bacc
    import concourse.mybir as mybir
    import concourse.tile as tile

    bf16 = mybir.dt.bfloat16
    nc = bacc.Bacc(
        "TRN2", target_bir_lowering=False, debug=False, num_devices=N_CORES
    )
    x_d = nc.dram_tensor("x", (n_imgs * 512, 512), bf16, kind="ExternalInput")
    c_d = nc.dram_tensor("cst", (P, 256), bf16, kind="ExternalInput")
    o_d = nc.dram_tensor("out", (4 * n_imgs * 512, 512), bf16, kind="ExternalOutput")

    with tile.TileContext(nc) as tc:
        with ExitStack() as ctx:
            _body(ctx, tc, o_d.ap(), x_d.ap(), c_d.ap(), n_imgs)
    nc.compile()
    _BUILT[key] = nc
    return nc


def _run(x, trace=False):
    """x: (32, 3, 512, 512) float32. Returns (out, exec_time_ns)."""
    import ml_dtypes

    from concourse import bass_utils

    nc = _build(IMGS)
    consts = _consts()
    xb = x.astype(ml_dtypes.bfloat16)
    in_maps = []
    for k in range(N_CORES):
        xs = xb[k * B_PER_CORE : (k + 1) * B_PER_CORE].reshape(IMGS * 512, 512)
        in_maps.append({"x": np.ascontiguousarray(xs), "cst": consts})
    res = bass_utils.run_bass_kernel_spmd(
        nc, in_maps, core_ids=list(range(N_CORES)), trace=trace
    )
    outs = []
    for k in range(N_CORES):
        o = res.results[k]["out"].astype(np.float32)
        outs.append(o.reshape(4, B_PER_CORE, DCH, H, W))
    full = np.concatenate(outs, axis=1)  # (4, 32, 3, 512, 512)
    return full, res.exec_time_ns


def kernel(**inputs) -> np.ndarray:
    x = np.ascontiguousarray(np.asarray(inputs["x"], dtype=np.float32))
    assert x.shape == (FULL_B, DCH, H, W), x.shape
    out, _ = _run(x, trace=False)
    return out


# revision 6
# speedup vs baseline: 1.2810x; 1.2810x over previous
"""Trainium2 Bass kernel for the blocked-DCT corner-mask layer.

Math: for each 8x8 block B of the image, the reference computes
    coeffs = D^T B D        (2D DCT-II)
    out_c  = D (coeffs * mask_c) D^T   for 4 corner masks c
Each mask is an outer product of half-indicators, so with
    L = D[:, :4] @ D[:, :4].T   (symmetric projection),  H = I - L
the whole pipeline collapses to
    out_0 = L B L,  out_1 = L B H,  out_2 = H B L,  out_3 = H B H.

Per-8-row/8-col application over a full 512x512 image is multiplication by
the 128x128 block-diagonal BDL = blockdiag(L x 16) (symmetric) on either
side.  On-chip per [128, 512] tile X:
    A-mm  c: lhsT = X[:, 128c:128c+128]  -> A(c) = [R^T(c) | RH^T(c)],
             R = BDL @ X, RH = BDH @ X            (PE, rhs=[BDL|BDH], N=256)
    out-mm c: lhsT = R^T(c)  -> [O0(c) | O1(c)];
              lhsT = RH^T(c) -> [O2(c) | O3(c)]   (PE, N=256)

All HBM I/O and matmul operands are bf16 (tolerance is 2e-2; bf16 costs
~3e-3): halves both the DMA bytes (the f32 bottleneck) and PE time.
The f32->bf16 cast rides the PSUM->SBUF copies that are needed anyway.

Pipelining: PSUM is two rings of [128,1024] f32 (2 banks) x 2 bufs --
front ring (A) and back ring (p01/p23 alternating), so tile i+1's
matmuls overlap tile i's drain copies.  Each PSUM drain is a single
wide copy op (DVE: A-head + [O0|O1]; ACT: A-tail + [O2|O3]), with the
de-interleave folded into the copy access pattern.  Outputs stage into
per-corner [128, 512*4] buffers covering a whole image plane, so each
in/out DMA moves 512 KB (4 tiles) in one descriptor batch.

Sharding: data-parallel over batch, 4 batches (12 images) per core.
"""

import numpy as np

FULL_B, DCH, H, W = 32, 3, 512, 512
N_CORES = 8
B_PER_CORE = FULL_B // N_CORES       # 4
IMGS = B_PER_CORE * DCH              # 12 images per core
P = 128
SPLIT = 416  # DVE/ACT balance point for the A-copy (DVE 1.042, ACT 0.833 ns/col)

_BUILT = {}


def _consts() -> np.ndarray:
    """[128, 256] = [BDL | BDH] constants, computed in float64 -> bf16."""
    import ml_dtypes

    N = 8
    x = np.arange(N, dtype=np.float64)[:, None]
    u = np.arange(N, dtype=np.float64)[None, :]
    alpha = np.full(N, np.sqrt(2.0 / N))
    alpha[0] = np.sqrt(1.0 / N)
    D = alpha[None, :] * np.cos(np.pi * u * (2.0 * x + 1.0) / (2.0 * N))
    L = D[:, :4] @ D[:, :4].T
    Hm = np.eye(N) - L
    BDL = np.kron(np.eye(16), L)
    BDH = np.kron(np.eye(16), Hm)
    cst = np.concatenate([BDL, BDH], axis=1)
    return np.ascontiguousarray(cst.astype(ml_dtypes.bfloat16))


def _img_ap(dram_ap, row0):
    """[128, 4, 512] view of one 512-row image plane: (t*128+p, w) -> (p, t, w)."""
    return dram_ap[row0 : row0 + 512, :].rearrange("(t p) w -> p t w", t=4)


def _body(ctx, tc, o_ap, x_ap, c_ap, n_imgs):
    import concourse.mybir as mybir

    nc = tc.nc
    f32 = mybir.dt.float32
    bf16 = mybir.dt.bfloat16

    cpool = ctx.enter_context(tc.tile_pool(name="const", bufs=1))
    cst = cpool.tile([P, 256], bf16)
    nc.sync.dma_start(cst[:], c_ap[:, :])
    BDLH = cst[:, 0:256]  # packed [BDL | BDH] rhs, N=256

    sb = ctx.enter_context(tc.tile_pool(name="sb", bufs=1))
    ps = ctx.enter_context(tc.tile_pool(name="ps", bufs=1, space="PSUM"))

    xbigs = {}
    obigs = {}

    def front(i):
        """per-image input DMA + row-transform matmuls A = x^T @ [BDL|BDH] + copy.

        A-mm for chunk c: lhsT = X[:, 128c:128c+128] (contraction over
        image rows) -> out [128 = col-in-chunk, 256] = [R^T(c) | RH^T(c)]
        where R = BDL @ X, RH = BDH @ X.  No identity transposes needed.
        """
        img, t = divmod(i, 4)
        if t == 0:
            xbig = sb.tile([P, 2048], bf16, tag="x", bufs=3, name=f"x_{img}")
            xv = xbig[:].rearrange("p (t w) -> p t w", t=4)
            nc.gpsimd.dma_start(xv, _img_ap(x_ap, img * 512))  # SWDGE ring
            xbigs[img] = xbig
        xbig = xbigs[img]

        a_ps = ps.tile([P, 1024], f32, tag="aps", bufs=2, name=f"aps_{i}")
        for c in range(4):
            nc.tensor.matmul(
                a_ps[:, 256 * c : 256 * (c + 1)],
                lhsT=xbig[:, 512 * t + 128 * c : 512 * t + 128 * (c + 1)],
                rhs=BDLH,
                start=True,
                stop=True,
            )
        # split the A drain across both engines (balanced by engine rate)
        a_sb = sb.tile([P, 1024], bf16, tag="as", bufs=4, name=f"a_{i}")
        nc.vector.tensor_copy(a_sb[:, 0:SPLIT], a_ps[:, 0:SPLIT])  # DVE
        nc.scalar.copy(a_sb[:, SPLIT:1024], a_ps[:, SPLIT:1024])  # ACT
        return a_sb

    def back_half(i, a_sb, half):
        """One back half-phase: 4 matmuls -> [128,1024] PSUM -> one wide
        de-interleaving copy into the per-image staging tiles -> (at t=3)
        two 512 KB output DMAs."""
        img, t = divmod(i, 4)
        a_v = a_sb[:].rearrange("p (c s l) -> p c s l", c=4, s=2, l=128)

        pb = ps.tile([P, 1024], f32, tag="pb", bufs=2, name=f"p{half}_{i}")
        for c in range(4):
            nc.tensor.matmul(
                pb[:, 256 * c : 256 * (c + 1)],
                lhsT=a_v[:, c, half, :],
                rhs=BDLH,
                start=True,
                stop=True,
            )  # [O_{2h}(c) | O_{2h+1}(c)]

        # rolling 4-section (512-row) staging slab per corner pair; the o23
        # pair is offset by 2 tiles so a 1 MB slab pair fires every 2 tiles,
        # alternating rings, instead of 2 MB bursts every 4th tile.
        delta = 2 * half
        q = (i + delta) % 4
        if q == 0 or i == 0:
            obigs[half] = sb.tile(
                [P, 4096], bf16, tag=f"o{half}", bufs=2, name=f"o{half}_{i}"
            )
        obig = obigs[half]
        # de-interleave (c s l) -> (s c l) while draining PSUM, one wide op
        src = pb[:].rearrange("p (c s l) -> p s c l", c=4, s=2, l=128)
        dst = obig[:].rearrange("p (s t c l) -> p t s c l", s=2, t=4, c=4, l=128)[
            :, q
        ]
        if half == 0:
            nc.vector.tensor_copy(dst, src)  # DVE
        else:
            nc.scalar.copy(dst, src)  # ACT

        if q == 3 or i == ntiles - 1:
            b0 = (i - q) * 128  # corner-plane row of slab section 0
            qlo = max(0, q - i)
            n = q + 1 - qlo
            ov = obig[:].rearrange("p (s t w) -> p s t w", s=2, t=4)
            eng = nc.sync if half == 0 else nc.scalar
            for s in range(2):
                ci = 2 * half + s
                r0 = ci * n_imgs * 512 + b0 + qlo * 128
                eng.dma_start(
                    o_ap[r0 : r0 + n * 128, :].rearrange("(t p) w -> p t w", t=n),
                    ov[:, s, qlo : q + 1],
                )

    # one-stage software skew: tile i's output stages are emitted after
    # tile i+1's front stage, keeping PE fed while PSUM banks drain
    ntiles = n_imgs * 4
    pending = None
    for i in range(ntiles):
        cch = front(i)
        if pending is not None:
            back_half(i - 1, pending, 0)
            back_half(i - 1, pending, 1)
        pending = cch
    back_half(ntiles - 1, pending, 0)
    back_half(ntiles - 1, pending, 1)


def _build(n_imgs=IMGS):
    key = n_imgs
    if key in _BUILT:
        return _BUILT[key]
    from contextlib import ExitStack

    import concourse.bacc as bacc
    import concourse.mybir as mybir
    import concourse.tile as tile

    bf16 = mybir.dt.bfloat16
    nc = bacc.Bacc(
        "TRN2", target_bir_lowering=False, debug=False, num_devices=N_CORES
    )
    x_d = nc.dram_tensor("x", (n_imgs * 512, 512), bf16, kind="ExternalInput")
    c_d = nc.dram_tensor("cst", (P, 256), bf16, kind="ExternalInput")
    o_d = nc.dram_tensor("out", (4 * n_imgs * 512, 512), bf16, kind="ExternalOutput")

    with tile.TileContext(nc) as tc:
        with ExitStack() as ctx:
            _body(ctx, tc, o_d.ap(), x_d.ap(), c_d.ap(), n_imgs)
    nc.compile()
    _BUILT[key] = nc
    return nc


def _run(x, trace=False):
    """x: (32, 3, 512, 512) float32. Returns (out, exec_time_ns)."""
    import ml_dtypes

    from concourse import bass_utils

    nc = _build(IMGS)
    consts = _consts()
    xb = x.astype(ml_dtypes.bfloat16)
    in_maps = []
    for k in range(N_CORES):
        xs = xb[k * B_PER_CORE : (k + 1) * B_PER_CORE].reshape(IMGS * 512, 512)
        in_maps.append({"x": np.ascontiguousarray(xs), "cst": consts})
    res = bass_utils.run_bass_kernel_spmd(
        nc, in_maps, core_ids=list(range(N_CORES)), trace=trace
    )
    outs = []
    for k in range(N_CORES):
        o = res.results[k]["out"].astype(np.float32)
        outs.append(o.reshape(4, B_PER_CORE, DCH, H, W))
    full = np.concatenate(outs, axis=1)  # (4, 32, 3, 512, 512)
    return full, res.exec_time_ns


def kernel(**inputs) -> np.ndarray:
    x = np.ascontiguousarray(np.asarray(inputs["x"], dtype=np.float32))
    assert x.shape == (FULL_B, DCH, H, W), x.shape
    out, _ = _run(x, trace=False)
    return out


# revision 18
# speedup vs baseline: 1.3532x; 1.0563x over previous
"""Trainium2 Bass kernel for the blocked-DCT corner-mask layer.

Math: for each 8x8 block B of the image, the reference computes
    coeffs = D^T B D        (2D DCT-II)
    out_c  = D (coeffs * mask_c) D^T   for 4 corner masks c
Each mask is an outer product of half-indicators, so with
    L = D[:, :4] @ D[:, :4].T   (symmetric projection),  H = I - L
the whole pipeline collapses to
    out_0 = L B L,  out_1 = L B H,  out_2 = H B L,  out_3 = H B H.

Per-8-row/8-col application over a full 512x512 image is multiplication by
the 128x128 block-diagonal BDL = blockdiag(L x 16) (symmetric) on either
side.  On-chip per [128, 512] tile X:
    A-mm  c: lhsT = X[:, 128c:128c+128]  -> A(c) = [R^T(c) | RH^T(c)],
             R = BDL @ X, RH = BDH @ X            (PE, rhs=[BDL|BDH], N=256)
    out-mm c: lhsT = R^T(c)  -> [O0(c) | O1(c)];
              lhsT = RH^T(c) -> [O2(c) | O3(c)]   (PE, N=256)

All HBM I/O and matmul operands are bf16 (tolerance is 2e-2; bf16 costs
~3e-3): halves both the DMA bytes (the f32 bottleneck) and PE time.
The f32->bf16 cast rides the PSUM->SBUF copies that are needed anyway.

Pipelining: PSUM is two rings of [128,1024] f32 (2 banks) x 2 bufs --
front ring (A) and back ring (p01/p23 alternating), so tile i+1's
matmuls overlap tile i's drain copies.  Each PSUM drain is a single
wide copy op (DVE: A-head + [O0|O1]; ACT: A-tail + [O2|O3]), with the
de-interleave folded into the copy access pattern.

DMA discipline (the kernel is HBM-bound: 6.3 MB in + 25.2 MB out per
core ~= 88us at 358 GB/s): few large transfers, and each HWDGE/SWDGE
dma_start costs ~0.6-1.5us of descriptor-gen on its ISSUING sequencer,
so triggers only live on sequencers with no copy work -- inputs ride
per-image 512 KB SWDGE (gpsimd) transfers, outputs stage into rolling
[128, 4096] slabs per corner pair and fire as 512 KB transfers on the
sync HWDGE ring (o0/o1) and the gpsimd SWDGE ring (o2/o3).  The o23
pair is offset by 2 tiles so 1 MB fires every 2 tiles (no 2 MB bursts),
and the last image flushes in 2-section chunks so almost nothing
remains after the final copy.  Image 0 arrives as four separate
per-tile HWDGE tiles so the first matmul waits on 128 KB, not 512 KB
behind the SWDGE Q7 cold-start.

Sharding: data-parallel over batch, 4 batches (12 images) per core.
"""

import numpy as np

FULL_B, DCH, H, W = 32, 3, 512, 512
N_CORES = 8
B_PER_CORE = FULL_B // N_CORES       # 4
IMGS = B_PER_CORE * DCH              # 12 images per core
P = 128
SPLIT = 416  # DVE/ACT balance point for the A-copy (DVE 1.042, ACT 0.833 ns/col)

_BUILT = {}


def _consts() -> np.ndarray:
    """[128, 256] = [BDL | BDH] constants, computed in float64 -> bf16."""
    import ml_dtypes

    N = 8
    x = np.arange(N, dtype=np.float64)[:, None]
    u = np.arange(N, dtype=np.float64)[None, :]
    alpha = np.full(N, np.sqrt(2.0 / N))
    alpha[0] = np.sqrt(1.0 / N)
    D = alpha[None, :] * np.cos(np.pi * u * (2.0 * x + 1.0) / (2.0 * N))
    L = D[:, :4] @ D[:, :4].T
    Hm = np.eye(N) - L
    BDL = np.kron(np.eye(16), L)
    BDH = np.kron(np.eye(16), Hm)
    cst = np.concatenate([BDL, BDH], axis=1)
    return np.ascontiguousarray(cst.astype(ml_dtypes.bfloat16))


def _img_ap(dram_ap, row0):
    """[128, 4, 512] view of one 512-row image plane: (t*128+p, w) -> (p, t, w)."""
    return dram_ap[row0 : row0 + 512, :].rearrange("(t p) w -> p t w", t=4)


def _body(ctx, tc, o_ap, x_ap, c_ap, n_imgs):
    import concourse.mybir as mybir

    nc = tc.nc
    f32 = mybir.dt.float32
    bf16 = mybir.dt.bfloat16

    cpool = ctx.enter_context(tc.tile_pool(name="const", bufs=1))
    cst = cpool.tile([P, 256], bf16)
    nc.sync.dma_start(cst[:], c_ap[:, :])
    BDLH = cst[:, 0:256]  # packed [BDL | BDH] rhs, N=256

    sb = ctx.enter_context(tc.tile_pool(name="sb", bufs=1))
    ps = ctx.enter_context(tc.tile_pool(name="ps", bufs=1, space="PSUM"))

    xbigs = {}
    obigs = {}
    flushed = {}

    def front(i):
        """per-image input DMA + row-transform matmuls A = x^T @ [BDL|BDH] + copy.

        A-mm for chunk c: lhsT = X[:, 128c:128c+128] (contraction over
        image rows) -> out [128 = col-in-chunk, 256] = [R^T(c) | RH^T(c)]
        where R = BDL @ X, RH = BDH @ X.  No identity transposes needed.
        """
        img, t = divmod(i, 4)
        if img == 0:
            # image 0 arrives as four separate per-tile tiles on the (empty)
            # HWDGE rings, so the first matmul waits on one 128 KB transfer
            # (tile-granular dependency tracking), not a 512 KB SWDGE one
            # behind the Q7 cold-start.
            xtile = sb.tile([P, 512], bf16, tag="x0", bufs=4, name=f"x0_{t}")
            eng = nc.sync if t % 2 == 0 else nc.scalar
            eng.dma_start(xtile[:], x_ap[t * 128 : (t + 1) * 128, :])
            xoff = 0
        else:
            if t == 0:
                xbig = sb.tile([P, 2048], bf16, tag="x", bufs=6, name=f"x_{img}")
                xv = xbig[:].rearrange("p (t w) -> p t w", t=4)
                nc.gpsimd.dma_start(xv, _img_ap(x_ap, img * 512))  # SWDGE ring
                xbigs[img] = xbig
            xtile = xbigs[img]
            xoff = 512 * t

        a_ps = ps.tile([P, 1024], f32, tag="aps", bufs=2, name=f"aps_{i}")
        for c in range(4):
            nc.tensor.matmul(
                a_ps[:, 256 * c : 256 * (c + 1)],
                lhsT=xtile[:, xoff + 128 * c : xoff + 128 * (c + 1)],
                rhs=BDLH,
                start=True,
                stop=True,
            )
        # split the A drain across both engines (balanced by engine rate)
        a_sb = sb.tile([P, 1024], bf16, tag="as", bufs=4, name=f"a_{i}")
        nc.vector.tensor_copy(a_sb[:, 0:SPLIT], a_ps[:, 0:SPLIT])  # DVE
        nc.scalar.copy(a_sb[:, SPLIT:1024], a_ps[:, SPLIT:1024])  # ACT
        return a_sb

    def back_half(i, a_sb, half):
        """One back half-phase: 4 matmuls -> [128,1024] PSUM -> one wide
        de-interleaving copy into the per-image staging tiles -> (at t=3)
        two 512 KB output DMAs."""
        img, t = divmod(i, 4)
        a_v = a_sb[:].rearrange("p (c s l) -> p c s l", c=4, s=2, l=128)

        pb = ps.tile([P, 1024], f32, tag="pb", bufs=2, name=f"p{half}_{i}")
        for c in range(4):
            nc.tensor.matmul(
                pb[:, 256 * c : 256 * (c + 1)],
                lhsT=a_v[:, c, half, :],
                rhs=BDLH,
                start=True,
                stop=True,
            )  # [O_{2h}(c) | O_{2h+1}(c)]

        # rolling 4-section (512-row) staging slab per corner pair; the o23
        # pair is offset by 2 tiles so a 1 MB slab pair fires every 2 tiles,
        # alternating rings, instead of 2 MB bursts every 4th tile.
        delta = 2 * half
        q = (i + delta) % 4
        if q == 0 or i == 0:
            obigs[half] = sb.tile(
                [P, 4096], bf16, tag=f"o{half}", bufs=3, name=f"o{half}_{i}"
            )
            flushed[half] = q  # sections below q are phantom (pre-plane)
        obig = obigs[half]
        # de-interleave (c s l) -> (s c l) while draining PSUM, one wide op
        src = pb[:].rearrange("p (c s l) -> p s c l", c=4, s=2, l=128)
        dst = obig[:].rearrange("p (s t c l) -> p t s c l", s=2, t=4, c=4, l=128)[
            :, q
        ]
        if half == 0:
            nc.vector.tensor_copy(dst, src)  # DVE
        else:
            nc.scalar.copy(dst, src)  # ACT

        # fire a full slab at q==3; over the last image flush every 2
        # sections so little output DMA (and little per-DMA descriptor-gen
        # on the sequencer) remains after the final copy
        if (q == 3 or (i >= ntiles - 4 and q % 2 == 1)) and q >= flushed[half]:
            b0 = (i - q) * 128  # corner-plane row of slab section 0
            qlo = flushed[half]
            n = q + 1 - qlo
            flushed[half] = q + 1
            ov = obig[:].rearrange("p (s t w) -> p s t w", s=2, t=4)
            # o01 on the HWDGE sync ring; o23 on the SWDGE (gpsimd) ring so
            # the ~0.6-1us per-DMA descriptor-gen never lands on a sequencer
            # that also dispatches copies.
            eng = nc.sync if half == 0 else nc.gpsimd
            for s in range(2):
                ci = 2 * half + s
                r0 = ci * n_imgs * 512 + b0 + qlo * 128
                eng.dma_start(
                    o_ap[r0 : r0 + n * 128, :].rearrange("(t p) w -> p t w", t=n),
                    ov[:, s, qlo : q + 1],
                )

    # one-stage software skew: tile i's output stages are emitted after
    # tile i+1's front stage, keeping PE fed while PSUM banks drain
    ntiles = n_imgs * 4
    pending = None
    for i in range(ntiles):
        cch = front(i)
        if pending is not None:
            back_half(i - 1, pending, 0)
            back_half(i - 1, pending, 1)
        pending = cch
    back_half(ntiles - 1, pending, 0)
    back_half(ntiles - 1, pending, 1)


def _build(n_imgs=IMGS):
    key = n_imgs
    if key in _BUILT:
        return _BUILT[key]
    from contextlib import ExitStack

    import concourse.bacc as bacc
    import concourse.mybir as mybir
    import concourse.tile as tile

    bf16 = mybir.dt.bfloat16
    nc = bacc.Bacc(
        "TRN2", target_bir_lowering=False, debug=False, num_devices=N_CORES
    )
    x_d = nc.dram_tensor("x", (n_imgs * 512, 512), bf16, kind="ExternalInput")
    c_d = nc.dram_tensor("cst", (P, 256), bf16, kind="ExternalInput")
    o_d = nc.dram_tensor("out", (4 * n_imgs * 512, 512), bf16, kind="ExternalOutput")

    with tile.TileContext(nc) as tc:
        with ExitStack() as ctx:
            _body(ctx, tc, o_d.ap(), x_d.ap(), c_d.ap(), n_imgs)
    nc.compile()
    _BUILT[key] = nc
    return nc


def _run(x, trace=False):
    """x: (32, 3, 512, 512) float32. Returns (out, exec_time_ns)."""
    import ml_dtypes

    from concourse import bass_utils

    nc = _build(IMGS)
    consts = _consts()
    xb = x.astype(ml_dtypes.bfloat16)
    in_maps = []
    for k in range(N_CORES):
        xs = xb[k * B_PER_CORE : (k + 1) * B_PER_CORE].reshape(IMGS * 512, 512)
        in_maps.append({"x": np.ascontiguousarray(xs), "cst": consts})
    res = bass_utils.run_bass_kernel_spmd(
        nc, in_maps, core_ids=list(range(N_CORES)), trace=trace
    )
    outs = []
    for k in range(N_CORES):
        o = res.results[k]["out"].astype(np.float32)
        outs.append(o.reshape(4, B_PER_CORE, DCH, H, W))
    full = np.concatenate(outs, axis=1)  # (4, 32, 3, 512, 512)
    return full, res.exec_time_ns


def kernel(**inputs) -> np.ndarray:
    x = np.ascontiguousarray(np.asarray(inputs["x"], dtype=np.float32))
    assert x.shape == (FULL_B, DCH, H, W), x.shape
    out, _ = _run(x, trace=False)
    return out


# revision 21
# speedup vs baseline: 1.3657x; 1.0092x over previous
"""Trainium2 Bass kernel for the blocked-DCT corner-mask layer.

Math: for each 8x8 block B of the image, the reference computes
    coeffs = D^T B D        (2D DCT-II)
    out_c  = D (coeffs * mask_c) D^T   for 4 corner masks c
Each mask is an outer product of half-indicators, so with
    L = D[:, :4] @ D[:, :4].T   (symmetric projection),  H = I - L
the whole pipeline collapses to
    out_0 = L B L,  out_1 = L B H,  out_2 = H B L,  out_3 = H B H.

Per-8-row/8-col application over a full 512x512 image is multiplication by
the 128x128 block-diagonal BDL = blockdiag(L x 16) (symmetric) on either
side.  On-chip per [128, 512] tile X:
    A-mm  c: lhsT = X[:, 128c:128c+128]  -> A(c) = [R^T(c) | RH^T(c)],
             R = BDL @ X, RH = BDH @ X            (PE, rhs=[BDL|BDH], N=256)
    out-mm c: lhsT = R^T(c)  -> [O0(c) | O1(c)];
              lhsT = RH^T(c) -> [O2(c) | O3(c)]   (PE, N=256)

All HBM I/O and matmul operands are bf16 (tolerance is 2e-2; bf16 costs
~3e-3): halves both the DMA bytes (the f32 bottleneck) and PE time.
The f32->bf16 cast rides the PSUM->SBUF copies that are needed anyway.

Pipelining: PSUM is two rings of [128,1024] f32 (2 banks) x 2 bufs --
front ring (A) and back ring (p01/p23 alternating), so tile i+1's
matmuls overlap tile i's drain copies.  Each PSUM drain is a single
wide copy op (DVE: A-head + [O0|O1]; ACT: A-tail + [O2|O3]), with the
de-interleave folded into the copy access pattern.

DMA discipline (the kernel is HBM-bound: 6.3 MB in + 25.2 MB out per
core ~= 88us at 358 GB/s): few large transfers, and each HWDGE/SWDGE
dma_start costs ~0.6-1.5us of descriptor-gen on its ISSUING sequencer,
so triggers only live on sequencers with no copy work -- inputs ride
per-image 512 KB SWDGE (gpsimd) transfers, outputs stage into rolling
[128, 4096] slabs per corner pair and fire as 512 KB transfers on the
sync HWDGE ring (o0/o1) and the gpsimd SWDGE ring (o2/o3).  The o23
pair is offset by 2 tiles so 1 MB fires every 2 tiles (no 2 MB bursts),
and the last image flushes in 2-section chunks so almost nothing
remains after the final copy.  Image 0 arrives as four separate
per-tile HWDGE tiles so the first matmul waits on 128 KB, not 512 KB
behind the SWDGE Q7 cold-start.

Sharding: data-parallel over batch, 4 batches (12 images) per core.
"""

import numpy as np

FULL_B, DCH, H, W = 32, 3, 512, 512
N_CORES = 8
B_PER_CORE = FULL_B // N_CORES       # 4
IMGS = B_PER_CORE * DCH              # 12 images per core
P = 128
SPLIT = 416  # DVE/ACT balance point for the A-copy (DVE 1.042, ACT 0.833 ns/col)

_BUILT = {}


def _consts() -> np.ndarray:
    """[128, 256] = [BDL | BDH] constants, computed in float64 -> bf16."""
    import ml_dtypes

    N = 8
    x = np.arange(N, dtype=np.float64)[:, None]
    u = np.arange(N, dtype=np.float64)[None, :]
    alpha = np.full(N, np.sqrt(2.0 / N))
    alpha[0] = np.sqrt(1.0 / N)
    D = alpha[None, :] * np.cos(np.pi * u * (2.0 * x + 1.0) / (2.0 * N))
    L = D[:, :4] @ D[:, :4].T
    Hm = np.eye(N) - L
    BDL = np.kron(np.eye(16), L)
    BDH = np.kron(np.eye(16), Hm)
    cst = np.concatenate([BDL, BDH], axis=1)
    return np.ascontiguousarray(cst.astype(ml_dtypes.bfloat16))


def _img_ap(dram_ap, row0):
    """[128, 4, 512] view of one 512-row image plane: (t*128+p, w) -> (p, t, w)."""
    return dram_ap[row0 : row0 + 512, :].rearrange("(t p) w -> p t w", t=4)


def _body(ctx, tc, o_ap, x_ap, c_ap, n_imgs):
    import concourse.mybir as mybir

    nc = tc.nc
    f32 = mybir.dt.float32
    bf16 = mybir.dt.bfloat16

    cpool = ctx.enter_context(tc.tile_pool(name="const", bufs=1))
    cst = cpool.tile([P, 256], bf16)
    nc.sync.dma_start(cst[:], c_ap[:, :])
    BDLH = cst[:, 0:256]  # packed [BDL | BDH] rhs, N=256

    sb = ctx.enter_context(tc.tile_pool(name="sb", bufs=1))
    ps = ctx.enter_context(tc.tile_pool(name="ps", bufs=1, space="PSUM"))

    xbigs = {}
    obigs = {}
    flushed = {}

    PREFETCH = 3  # images of input fetched ahead of compute

    def fetch(img):
        xbig = sb.tile([P, 2048], bf16, tag="x", bufs=6, name=f"x_{img}")
        xv = xbig[:].rearrange("p (t w) -> p t w", t=4)
        nc.gpsimd.dma_start(xv, _img_ap(x_ap, img * 512))  # SWDGE ring
        xbigs[img] = xbig

    def front(i):
        """per-image input DMA + row-transform matmuls A = x^T @ [BDL|BDH] + copy.

        A-mm for chunk c: lhsT = X[:, 128c:128c+128] (contraction over
        image rows) -> out [128 = col-in-chunk, 256] = [R^T(c) | RH^T(c)]
        where R = BDL @ X, RH = BDH @ X.  No identity transposes needed.
        """
        img, t = divmod(i, 4)
        if img == 0:
            # image 0 arrives as four separate per-tile tiles on the (empty)
            # HWDGE rings, so the first matmul waits on one 128 KB transfer
            # (tile-granular dependency tracking), not a 512 KB SWDGE one
            # behind the Q7 cold-start.
            xtile = sb.tile([P, 512], bf16, tag="x0", bufs=4, name=f"x0_{t}")
            eng = nc.sync if t % 2 == 0 else nc.scalar
            eng.dma_start(xtile[:], x_ap[t * 128 : (t + 1) * 128, :])
            if t == 0:
                # issue images 1..PREFETCH right away: the per-image SWDGE
                # fetch otherwise starts only when that image's compute does,
                # which surfaces as an ~every-image DMA-eligibility dip once
                # the output backlog drains late in the run
                for j in range(1, min(PREFETCH + 1, n_imgs)):
                    fetch(j)
            xoff = 0
        else:
            if t == 0 and img + PREFETCH < n_imgs:
                fetch(img + PREFETCH)
            xtile = xbigs[img]
            xoff = 512 * t

        a_ps = ps.tile([P, 1024], f32, tag="aps", bufs=2, name=f"aps_{i}")
        for c in range(4):
            nc.tensor.matmul(
                a_ps[:, 256 * c : 256 * (c + 1)],
                lhsT=xtile[:, xoff + 128 * c : xoff + 128 * (c + 1)],
                rhs=BDLH,
                start=True,
                stop=True,
            )
        # split the A drain across both engines (balanced by engine rate)
        a_sb = sb.tile([P, 1024], bf16, tag="as", bufs=4, name=f"a_{i}")
        nc.vector.tensor_copy(a_sb[:, 0:SPLIT], a_ps[:, 0:SPLIT])  # DVE
        nc.scalar.copy(a_sb[:, SPLIT:1024], a_ps[:, SPLIT:1024])  # ACT
        return a_sb

    def back_half(i, a_sb, half):
        """One back half-phase: 4 matmuls -> [128,1024] PSUM -> one wide
        de-interleaving copy into the per-image staging tiles -> (at t=3)
        two 512 KB output DMAs."""
        img, t = divmod(i, 4)
        a_v = a_sb[:].rearrange("p (c s l) -> p c s l", c=4, s=2, l=128)

        pb = ps.tile([P, 1024], f32, tag="pb", bufs=2, name=f"p{half}_{i}")
        for c in range(4):
            nc.tensor.matmul(
                pb[:, 256 * c : 256 * (c + 1)],
                lhsT=a_v[:, c, half, :],
                rhs=BDLH,
                start=True,
                stop=True,
            )  # [O_{2h}(c) | O_{2h+1}(c)]

        # rolling 4-section (512-row) staging slab per corner pair; the o23
        # pair is offset by 2 tiles so a 1 MB slab pair fires every 2 tiles,
        # alternating rings, instead of 2 MB bursts every 4th tile.
        delta = 2 * half
        q = (i + delta) % 4
        if q == 0 or i == 0:
            obigs[half] = sb.tile(
                [P, 4096], bf16, tag=f"o{half}", bufs=3, name=f"o{half}_{i}"
            )
            flushed[half] = q  # sections below q are phantom (pre-plane)
        obig = obigs[half]
        # de-interleave (c s l) -> (s c l) while draining PSUM, one wide op
        src = pb[:].rearrange("p (c s l) -> p s c l", c=4, s=2, l=128)
        dst = obig[:].rearrange("p (s t c l) -> p t s c l", s=2, t=4, c=4, l=128)[
            :, q
        ]
        if half == 0:
            nc.vector.tensor_copy(dst, src)  # DVE
        else:
            nc.scalar.copy(dst, src)  # ACT

        # fire a full slab at q==3; over the last image flush every 2
        # sections so little output DMA (and little per-DMA descriptor-gen
        # on the sequencer) remains after the final copy
        if (q == 3 or (i >= ntiles - 8 and q % 2 == 1)) and q >= flushed[half]:
            b0 = (i - q) * 128  # corner-plane row of slab section 0
            qlo = flushed[half]
            n = q + 1 - qlo
            flushed[half] = q + 1
            ov = obig[:].rearrange("p (s t w) -> p s t w", s=2, t=4)
            # o01 on the HWDGE sync ring; o23 on the SWDGE (gpsimd) ring so
            # the ~0.6-1us per-DMA descriptor-gen never lands on a sequencer
            # that also dispatches copies.
            eng = nc.sync if half == 0 else nc.gpsimd
            for s in range(2):
                ci = 2 * half + s
                r0 = ci * n_imgs * 512 + b0 + qlo * 128
                eng.dma_start(
                    o_ap[r0 : r0 + n * 128, :].rearrange("(t p) w -> p t w", t=n),
                    ov[:, s, qlo : q + 1],
                )

    # one-stage software skew: tile i's output stages are emitted after
    # tile i+1's front stage, keeping PE fed while PSUM banks drain
    ntiles = n_imgs * 4
    pending = None
    for i in range(ntiles):
        cch = front(i)
        if pending is not None:
            back_half(i - 1, pending, 0)
            back_half(i - 1, pending, 1)
        pending = cch
    back_half(ntiles - 1, pending, 0)
    back_half(ntiles - 1, pending, 1)


def _build(n_imgs=IMGS):
    key = n_imgs
    if key in _BUILT:
        return _BUILT[key]
    from contextlib import ExitStack

    import concourse.bacc as bacc
    import concourse.mybir as mybir
    import concourse.tile as tile

    bf16 = mybir.dt.bfloat16
    nc = bacc.Bacc(
        "TRN2", target_bir_lowering=False, debug=False, num_devices=N_CORES
    )
    x_d = nc.dram_tensor("x", (n_imgs * 512, 512), bf16, kind="ExternalInput")
    c_d = nc.dram_tensor("cst", (P, 256), bf16, kind="ExternalInput")
    o_d = nc.dram_tensor("out", (4 * n_imgs * 512, 512), bf16, kind="ExternalOutput")

    with tile.TileContext(nc) as tc:
        with ExitStack() as ctx:
            _body(ctx, tc, o_d.ap(), x_d.ap(), c_d.ap(), n_imgs)
    nc.compile()
    _BUILT[key] = nc
    return nc


def _run(x, trace=False):
    """x: (32, 3, 512, 512) float32. Returns (out, exec_time_ns)."""
    import ml_dtypes

    from concourse import bass_utils

    nc = _build(IMGS)
    consts = _consts()
    xb = x.astype(ml_dtypes.bfloat16)
    in_maps = []
    for k in range(N_CORES):
        xs = xb[k * B_PER_CORE : (k + 1) * B_PER_CORE].reshape(IMGS * 512, 512)
        in_maps.append({"x": np.ascontiguousarray(xs), "cst": consts})
    res = bass_utils.run_bass_kernel_spmd(
        nc, in_maps, core_ids=list(range(N_CORES)), trace=trace
    )
    outs = []
    for k in range(N_CORES):
        o = res.results[k]["out"].astype(np.float32)
        outs.append(o.reshape(4, B_PER_CORE, DCH, H, W))
    full = np.concatenate(outs, axis=1)  # (4, 32, 3, 512, 512)
    return full, res.exec_time_ns


def kernel(**inputs) -> np.ndarray:
    x = np.ascontiguousarray(np.asarray(inputs["x"], dtype=np.float32))
    assert x.shape == (FULL_B, DCH, H, W), x.shape
    out, _ = _run(x, trace=False)
    return out
